# revision 11
# baseline (speedup 1.0000x reference)
"""Multi-head attention (B=2, S=2048, D=1024, H=16, Dk=64) on 8 TRN2 cores.

Sharding: tensor-parallel on heads — 2 heads (dh=128 columns of the QKV
projections) per core.  Each core:
  1. projects qT/kT/vT = (W_slice.T @ x.T) for its 2 heads    [128, 4096]
  2. transposes vT into per-(b,h) [j, d] blocks with an appended
     ones-column (so P@V_aug also yields the softmax row-sums)
  3. scoresT = kT.T-stationary matmul -> pT = exp(scoresT/8) in [j, i]
     layout, PV accumulates oT_aug = [o_unnorm ; rowsums] in PSUM
  4. normalizes via PE-broadcast of 1/rowsum
  5. partialT = Wo_slice.T @ oT                               [1024, 4096]
Host sums the 8 partialT outputs, adds bo, and transposes back.

All matmuls use float32r (full-rate fp32) with fp32 PSUM accumulation.
"""

import numpy as np

D = 1024
NTOK = 4096  # B * S
B = 2
S = 2048
DH = 128  # head-dim block per core (2 heads x 64)
N_CORES = 8

_CACHE = {}


def _build_nc(mm_dtype="float32r"):
    import concourse.bacc as bacc
    import concourse.mybir as mybir
    import concourse.tile as tile

    dt = mybir.dt
    f32 = dt.float32
    mmdt = getattr(dt, mm_dtype)
    AF = mybir.ActivationFunctionType

    def mm(ap):
        return ap

    nc = bacc.Bacc("TRN2", target_bir_lowering=False, debug=False)

    xq = nc.dram_tensor("xq", [D, NTOK], mmdt, kind="ExternalInput").ap()
    xk = nc.dram_tensor("xk", [D, NTOK], mmdt, kind="ExternalInput").ap()
    xv = nc.dram_tensor("xv", [D, NTOK], mmdt, kind="ExternalInput").ap()
    wq = nc.dram_tensor("wq", [128, D], mmdt, kind="ExternalInput").ap()
    wk = nc.dram_tensor("wk", [128, D], mmdt, kind="ExternalInput").ap()
    wv = nc.dram_tensor("wv", [128, D], mmdt, kind="ExternalInput").ap()
    wo = nc.dram_tensor("wo", [128, D], mmdt, kind="ExternalInput").ap()
    bq = nc.dram_tensor("bq", [1, 128], mmdt, kind="ExternalInput").ap()
    bk = nc.dram_tensor("bk", [1, 128], mmdt, kind="ExternalInput").ap()
    bv = nc.dram_tensor("bv", [1, 128], mmdt, kind="ExternalInput").ap()
    c_ident = nc.dram_tensor("c_ident", [128, 64], mmdt, kind="ExternalInput").ap()
    c_ones512 = nc.dram_tensor("c_ones512", [1, 512], mmdt, kind="ExternalInput").ap()
    c_ones64 = nc.dram_tensor("c_ones64", [1, 64], mmdt, kind="ExternalInput").ap()
    c_onescol = nc.dram_tensor("c_onescol", [128, 1], mmdt, kind="ExternalInput").ap()
    pout = nc.dram_tensor("pout", [D, NTOK], f32, kind="ExternalOutput").ap()

    with tile.TileContext(nc) as tc:
        from contextlib import ExitStack

        with ExitStack() as stk:
            const = stk.enter_context(tc.tile_pool(name="const", bufs=1))
            wpool = stk.enter_context(tc.tile_pool(name="w", bufs=1))
            big = stk.enter_context(tc.tile_pool(name="big", bufs=1))
            xpool = stk.enter_context(tc.tile_pool(name="xt", bufs=3))

            # ---- constants (DMA'd: memset can't emit f32r) ----
            # ident: [128, 64] with a 64x64 identity in each partition half,
            # so the transpose's rhs sits at the same base partition as lhsT.
            ident = const.tile([128, 64], mmdt)
            nc.sync.dma_start(out=ident, in_=c_ident)
            ones_row = const.tile([1, 512], mmdt)
            nc.sync.dma_start(out=ones_row, in_=c_ones512)
            ones64 = const.tile([1, 64], mmdt)
            nc.sync.dma_start(out=ones64, in_=c_ones64)

            # ---- weights / biases ----
            wq_sb = wpool.tile([128, D], mmdt)
            wk_sb = wpool.tile([128, D], mmdt)
            wv_sb = wpool.tile([128, D], mmdt)
            wo_sb = wpool.tile([128, D], mmdt)
            nc.sync.dma_start(out=wq_sb, in_=wq)
            nc.sync.dma_start(out=wk_sb, in_=wk)
            nc.sync.dma_start(out=wv_sb, in_=wv)
            nc.sync.dma_start(out=wo_sb, in_=wo)
            bq_sb = const.tile([1, 128], mmdt)
            bk_sb = const.tile([1, 128], mmdt)
            bv_sb = const.tile([1, 128], mmdt)
            nc.sync.dma_start(out=bq_sb, in_=bq)
            nc.sync.dma_start(out=bk_sb, in_=bk)
            nc.sync.dma_start(out=bv_sb, in_=bv)

            # ---- persistent activations ----
            qT = big.tile([128, NTOK], mmdt)  # [dh, tok]
            kT = big.tile([128, NTOK], mmdt)
            vT = big.tile([128, NTOK], mmdt)
            v_sb = big.tile([128, 4 * 16 * 65], mmdt)  # [j, (b,h)*jt*(64+1)]
            oT = big.tile([128, NTOK], mmdt)  # [dh, tok] normalized context

            # ones columns of v_sb via broadcast DMA (v blocks overwritten later)
            import concourse.bass as bass_mod

            v_r0 = v_sb.rearrange("p (t c) -> p t c", c=65)
            ones_bcast = bass_mod.AP(
                tensor=c_onescol.tensor,
                offset=c_onescol.offset,
                ap=[[1, 128], [0, 64], [1, 1]],
            )
            nc.sync.dma_start(out=v_r0[:, :, 64:65], in_=ones_bcast)

            # =========== Phase P: projections qT/kT/vT ===========
            with tc.tile_pool(name="pp", bufs=8, space="PSUM") as pp:
                for x_dram, w_sb, b_sb, dst in (
                    (xq, wq_sb, bq_sb, qT),
                    (xk, wk_sb, bk_sb, kT),
                    (xv, wv_sb, bv_sb, vT),
                ):
                    for b in range(2):
                        acc = [pp.tile([128, 512], f32, tag="pp", name=f"acc{b}_{nn}") for nn in range(4)]
                        for kk in range(8):
                            x_t = xpool.tile([128, 2048], mmdt, tag="xt")
                            nc.sync.dma_start(
                                out=x_t,
                                in_=x_dram[
                                    kk * 128 : (kk + 1) * 128,
                                    b * 2048 : (b + 1) * 2048,
                                ],
                            )
                            for nn in range(4):
                                nc.tensor.matmul(
                                    acc[nn],
                                    lhsT=mm(w_sb[:, kk * 128 : (kk + 1) * 128]),
                                    rhs=mm(x_t[:, nn * 512 : (nn + 1) * 512]),
                                    start=(kk == 0),
                                    stop=False,
                                )
                        for nn in range(4):
                            # bias: acc += b_sb.T @ ones  (adds bias to each col)
                            nc.tensor.matmul(
                                acc[nn],
                                lhsT=mm(b_sb),
                                rhs=mm(ones_row),
                                start=False,
                                stop=True,
                            )
                        for nn in range(4):
                            col = b * 2048 + nn * 512
                            eng = nc.scalar.copy if nn % 2 == 0 else nc.vector.tensor_copy
                            eng(dst[:, col : col + 512], acc[nn])

            # =========== Phase T: transpose vT -> v_sb blocks ===========
            v_r = v_sb.rearrange("p (t c) -> p t c", c=65)
            with tc.tile_pool(name="tp", bufs=3, space="PSUM") as tpp:
                for b in range(2):
                    for h in range(2):
                        bh = b * 2 + h
                        for g in range(4):  # groups of 4 j-tiles
                            tp = tpp.tile([128, 4 * 64], mmdt, tag="tp")
                            for u in range(4):
                                jb = g * 4 + u
                                nc.tensor.transpose(
                                    tp[:, u * 64 : (u + 1) * 64],
                                    vT[
                                        h * 64 : (h + 1) * 64,
                                        b * 2048 + jb * 128 : b * 2048 + (jb + 1) * 128,
                                    ],
                                    ident[h * 64 : (h + 1) * 64, :],
                                )
                            tp_r = tp.rearrange("p (t c) -> p t c", c=64)
                            nc.scalar.copy(
                                v_r[:, bh * 16 + g * 4 : bh * 16 + g * 4 + 4, 0:64],
                                tp_r,
                            )

            # =========== Phase A: attention ===========
            with (
                tc.tile_pool(name="sc", bufs=2, space="PSUM") as scp,
                tc.tile_pool(name="ops", bufs=2, space="PSUM") as opp,
                tc.tile_pool(name="pt", bufs=3) as ptp,
                tc.tile_pool(name="rs", bufs=2) as rsp,
            ):
                for b in range(2):
                    for h in range(2):
                        bh = b * 2 + h
                        for half in range(2):
                            i0 = b * 2048 + half * 1024
                            o_ps = opp.tile([65, 1024], f32, tag="ops")
                            for jt in range(16):
                                sc = scp.tile([128, 1024], f32, tag="sc")
                                for c in range(2):
                                    nc.tensor.matmul(
                                        sc[:, c * 512 : (c + 1) * 512],
                                        lhsT=mm(
                                            kT[
                                                h * 64 : (h + 1) * 64,
                                                b * 2048 + jt * 128 : b * 2048 + (jt + 1) * 128,
                                            ]
                                        ),
                                        rhs=mm(
                                            qT[
                                                h * 64 : (h + 1) * 64,
                                                i0 + c * 512 : i0 + (c + 1) * 512,
                                            ]
                                        ),
                                        start=True,
                                        stop=True,
                                    )
                                pt = ptp.tile([128, 1024], mmdt, tag="pt")
                                nc.scalar.activation(pt, sc, AF.Exp, scale=0.125)
                                for c in range(2):
                                    nc.tensor.matmul(
                                        o_ps[:, c * 512 : (c + 1) * 512],
                                        lhsT=mm(
                                            v_sb[:, (bh * 16 + jt) * 65 : (bh * 16 + jt + 1) * 65]
                                        ),
                                        rhs=mm(pt[:, c * 512 : (c + 1) * 512]),
                                        start=(jt == 0),
                                        stop=(jt == 15),
                                    )
                            # normalization: oT[:, cols] = o_unnorm * (1/rowsum)
                            rinv = rsp.tile([1, 1024], mmdt, tag="rinv")
                            with nc.allow_low_precision(reason="tf32 rinv is plenty"):
                                nc.vector.reciprocal(rinv, o_ps[64:65, :])
                            Rp = scp.tile([128, 1024], f32, tag="sc")
                            for c in range(2):
                                nc.tensor.matmul(
                                    Rp[0:64, c * 512 : (c + 1) * 512],
                                    lhsT=mm(ones64),
                                    rhs=mm(rinv[:, c * 512 : (c + 1) * 512]),
                                    start=True,
                                    stop=True,
                                )
                            Rs = rsp.tile([64, 1024], f32, tag="rs")
                            nc.vector.tensor_copy(Rs, Rp[0:64, :])
                            nc.vector.tensor_mul(
                                oT[h * 64 : (h + 1) * 64, i0 : i0 + 1024],
                                o_ps[0:64, :],
                                Rs,
                            )

            # =========== Phase O: output projection ===========
            with (
                tc.tile_pool(name="op", bufs=4, space="PSUM") as opj,
                tc.tile_pool(name="st", bufs=4) as stp,
            ):
                for c8 in range(8):
                    for dt_ in range(8):
                        op = opj.tile([128, 512], f32, tag="op")
                        nc.tensor.matmul(
                            op,
                            lhsT=mm(wo_sb[:, dt_ * 128 : (dt_ + 1) * 128]),
                            rhs=mm(oT[:, c8 * 512 : (c8 + 1) * 512]),
                            start=True,
                            stop=True,
                        )
                        st = stp.tile([128, 512], f32, tag="st")
                        eng = nc.scalar.copy if dt_ % 2 == 0 else nc.vector.tensor_copy
                        eng(st, op)
                        nc.sync.dma_start(
                            out=pout[
                                dt_ * 128 : (dt_ + 1) * 128,
                                c8 * 512 : (c8 + 1) * 512,
                            ],
                            in_=st,
                        )

    nc.compile()
    return nc


MM_DTYPE = "float32r"


def _get_nc():
    key = ("nc", MM_DTYPE)
    if key not in _CACHE:
        _CACHE[key] = _build_nc(MM_DTYPE)
    return _CACHE[key]


def _ensure_ntff_hook():
    """Register the NTFF profile hook module if the image lacks it."""
    import sys
    import types

    if "antenv.axon_hooks" in sys.modules:
        return
    try:
        from trn_agent_boot.trn_boot import _ntff_profile_via_ctypes
    except Exception:
        return
    hook = None
    try:
        hook = _ntff_profile_via_ctypes("/opt/axon/libaxon_pjrt.so")
    except Exception:
        hook = None
    mod = types.ModuleType("antenv.axon_hooks")
    mod._hook = hook
    mod.get_axon_ntff_profile_hook = lambda: mod._hook
    mod.set_axon_ntff_profile_hook = lambda h: setattr(mod, "_hook", h)
    sys.modules["antenv.axon_hooks"] = mod


def _run(inputs, trace=False):
    from concourse import bass_utils

    if trace:
        _ensure_ntff_hook()

    nc = _get_nc()
    query = np.asarray(inputs["query"], np.float32)
    key = np.asarray(inputs["key"], np.float32)
    value = np.asarray(inputs["value"], np.float32)
    Wq = np.asarray(inputs["Wq"], np.float32)
    Wk = np.asarray(inputs["Wk"], np.float32)
    Wv = np.asarray(inputs["Wv"], np.float32)
    Wo = np.asarray(inputs["Wo"], np.float32)
    bq = np.asarray(inputs["bq"], np.float32)
    bk = np.asarray(inputs["bk"], np.float32)
    bv = np.asarray(inputs["bv"], np.float32)
    bo = np.asarray(inputs["bo"], np.float32)

    if MM_DTYPE == "bfloat16":
        import ml_dtypes

        ext_dt = ml_dtypes.bfloat16
    else:
        ext_dt = np.float32

    xqT = np.ascontiguousarray(query.reshape(NTOK, D).T.astype(ext_dt))
    xkT = np.ascontiguousarray(key.reshape(NTOK, D).T.astype(ext_dt))
    xvT = np.ascontiguousarray(value.reshape(NTOK, D).T.astype(ext_dt))

    def pack_w(Wc):
        return np.ascontiguousarray(
            Wc.reshape(8, 128, 128).transpose(1, 0, 2).reshape(128, D).astype(ext_dt)
        )

    ident_np = np.zeros((128, 64), np.float32)
    ident_np[np.arange(64), np.arange(64)] = 1.0
    ident_np[64 + np.arange(64), np.arange(64)] = 1.0
    consts = {
        "c_ident": np.ascontiguousarray(ident_np.astype(ext_dt)),
        "c_ones512": np.ones((1, 512), ext_dt),
        "c_ones64": np.ones((1, 64), ext_dt),
        "c_onescol": np.ones((128, 1), ext_dt),
    }
    in_maps = []
    for c in range(N_CORES):
        sl = slice(c * 128, (c + 1) * 128)
        in_maps.append(
            {
                **consts,
                "xq": xqT,
                "xk": xkT,
                "xv": xvT,
                "wq": pack_w(Wq[:, sl]),
                "wk": pack_w(Wk[:, sl]),
                "wv": pack_w(Wv[:, sl]),
                "wo": np.ascontiguousarray(Wo[sl, :].astype(ext_dt)),
                "bq": np.ascontiguousarray(bq[sl].reshape(1, 128).astype(ext_dt)),
                "bk": np.ascontiguousarray(bk[sl].reshape(1, 128).astype(ext_dt)),
                "bv": np.ascontiguousarray(bv[sl].reshape(1, 128).astype(ext_dt)),
            }
        )

    res = bass_utils.run_bass_kernel_spmd(
        nc, in_maps, core_ids=list(range(N_CORES)), trace=trace
    )
    outT = np.zeros((D, NTOK), np.float64)
    for c in range(N_CORES):
        outT += np.asarray(res.results[c]["pout"], np.float64)
    out = (outT.T + bo.astype(np.float64)).astype(np.float32)
    return out.reshape(B, S, D), res


def kernel(**inputs):
    out, _ = _run(inputs, trace=False)
    return out
